# revision 8
# baseline (speedup 1.0000x reference)
"""Trainium2 Bass kernel for PVT-style spatial-reduction attention.

Reference computation (per batch element b, all fp32):
    q   = x @ q_w                             [N=4096, C=256], heads=8, hd=32
    xs  = conv4x4_stride4(x as [C,64,64]) + sr_b   -> [Nk=256, C]
    xs  = LayerNorm(xs) (over C)
    kv  = xs @ kv_w -> k, v                   [Nk, C] each
    att = softmax(q @ k^T / sqrt(hd))         per head
    out = (att @ v) @ proj_w + proj_b         [N, C]

Sharding: pure data parallel -- batch B=8, one batch element per NeuronCore,
no collectives. Host folds ln_w/ln_b/scale into the weights.

Device-side layout strategy (per core):
    xT  [C, N]   feature-major via PE transposes (fp32: no DMA transpose)
    qT  [C, N]   = q_w^T-contract (weights=q_w, rhs=xT)
    conv token-major [p, C] via 32 accumulated matmuls with strided patch
                 APs of xT as the stationary operand; +sr_b via K=1 matmul
    LN on free axis (bn_stats/bn_aggr); rsqrt = Exp(-0.5*Ln(var+eps))
                 (Ln/Exp share one ACT table set with softmax's Exp)
    kT  [hd*4, Nk] per head-group (feature-major), v token-major [Nk, C]
    scores^T [m, n] per head: K=32 matmuls, 4 heads row-tiled concurrently
                 (tile_position=(32h',0)); softmax needs no max-subtraction
                 (|scores| < ~0.7 by construction)
    exp via ACT on [128, 1024] PSUM tiles -> SBUF
    colsum: col-tiled M=1 ones-matmuls; reciprocal on DVE; broadcast to
                 [128, n] via 4 diagonal K=1 matmuls
    att@v: col-tiled M=32 matmuls accumulating over m
    proj token-major (weights = aoT chunks) so the output DMA is contiguous
"""

from contextlib import ExitStack

import numpy as np

import concourse.bass as bass
import concourse.tile as tile
from concourse import bacc
from concourse import mybir
from concourse.bass_utils import run_bass_kernel_spmd
from concourse.masks import make_identity

F32 = mybir.dt.float32
AF = mybir.ActivationFunctionType
ALU = mybir.AluOpType

N_CORES = 8
N, C = 4096, 256
HEADS, HD = 8, 32
NK = 256  # (64/4) * (64/4) patches
LN_EPS = 1e-5
NT = 512  # n-tile width for the attention loop
NNT = N // NT


def _emit(tc, x_d, qw_d, kvk_d, kvv_d, wconv_d, srb_d, projw_d, projb_d, out_d):
    nc = tc.nc

    with ExitStack() as top:
        consts = top.enter_context(tc.tile_pool(name="consts", bufs=1))
        ident = consts.tile([128, 128], F32)
        make_identity(nc, ident[:])
        ones_col = consts.tile([128, 1], F32)
        nc.gpsimd.memset(ones_col[:], 1.0)
        ones_row = consts.tile([128, 128], F32)
        nc.gpsimd.memset(ones_row[:], 1.0)
        eps_t = consts.tile([128, 1], F32)
        nc.gpsimd.memset(eps_t[:], LN_EPS)

        wpool = top.enter_context(tc.tile_pool(name="wts", bufs=1))

        def load_w(d, r0, tag):
            t = wpool.tile([128, C], F32, tag=tag, name=tag)
            nc.sync.dma_start(t[:], d[r0 : r0 + 128, :])
            return t

        qw_sb = [load_w(qw_d, ct * 128, f"qw{ct}") for ct in range(2)]
        kvk_sb = [load_w(kvk_d, ct * 128, f"kvk{ct}") for ct in range(2)]
        kvv_sb = [load_w(kvv_d, ct * 128, f"kvv{ct}") for ct in range(2)]
        projw_sb = [load_w(projw_d, ct * 128, f"pw{ct}") for ct in range(2)]
        srb_sb = wpool.tile([1, C], F32, tag="srb")
        nc.sync.dma_start(srb_sb[:], srb_d[:, :])
        projb_sb = wpool.tile([1, C], F32, tag="pb")
        nc.sync.dma_start(projb_sb[:], projb_d[:, :])

        persist = top.enter_context(tc.tile_pool(name="persist", bufs=1))
        qT_sb = [persist.tile([128, N], F32, tag=f"qT{g}", name=f"qT{g}") for g in range(2)]
        ln_sb = [persist.tile([128, C], F32, tag=f"ln{p}", name=f"ln{p}") for p in range(2)]
        lnT_sb = [persist.tile([128, NK], F32, tag=f"lnT{c}", name=f"lnT{c}") for c in range(2)]
        kT_sb = [persist.tile([128, NK], F32, tag=f"kT{g}", name=f"kT{g}") for g in range(2)]
        v_sb = [persist.tile([128, C], F32, tag=f"v{m}", name=f"v{m}") for m in range(2)]

        # ---------------- Phase A: xT, conv+LN, kT/v, qT ----------------
        with ExitStack() as pa:
            xTp = pa.enter_context(tc.tile_pool(name="xT", bufs=1))
            xT_sb = [xTp.tile([128, N], F32, tag=f"xT{c}", name=f"xT{c}") for c in range(2)]
            xl = pa.enter_context(tc.tile_pool(name="xload", bufs=4))
            pst = pa.enter_context(tc.tile_pool(name="ps_t", bufs=3, space="PSUM"))

            for quad in range(8):
                pss = [pst.tile([128, 512], F32, tag="pst", name=f"pst{c}") for c in range(2)]
                for k in range(4):
                    nb = quad * 4 + k
                    xt = xl.tile([128, C], F32, tag="x")
                    nc.sync.dma_start(xt[:], x_d[nb * 128 : (nb + 1) * 128, :])
                    for cb in range(2):
                        nc.tensor.matmul(
                            pss[cb][:, k * 128 : (k + 1) * 128],
                            xt[:, cb * 128 : (cb + 1) * 128],
                            ident[:],
                            is_transpose=True,
                            start=(k == 0),
                            stop=(k == 3),
                            skip_group_check=True,
                        )
                for cb in range(2):
                    nc.vector.tensor_copy(
                        xT_sb[cb][:, quad * 512 : (quad + 1) * 512], pss[cb][:]
                    )

            # conv (token-major) + LayerNorm
            wcp = pa.enter_context(tc.tile_pool(name="wconv", bufs=4))
            psc_p = pa.enter_context(tc.tile_pool(name="ps_c", bufs=2, space="PSUM"))
            small = pa.enter_context(tc.tile_pool(name="lnsmall", bufs=2))
            # xP[c, n'] with n' = i*256 + j*16 + a*4 + b (patch-major token
            # order) so conv patch weights are a single-strided AP.
            xP_sb = [xTp.tile([128, N], F32, tag=f"xP{c}", name=f"xP{c}") for c in range(2)]
            for ct in range(2):
                nc.vector.tensor_copy(
                    xP_sb[ct][:],
                    xT_sb[ct][:].rearrange(
                        "c (i a j b) -> c i j a b", i=16, a=4, j=16, b=4
                    ),
                )
            xT_r = [
                xP_sb[ct][:].rearrange("c (p s) -> c p s", p=256, s=16)
                for ct in range(2)
            ]
            for pb in range(2):
                psc = psc_p.tile([128, C], F32, tag="psA")
                idx = 0
                for ct in range(2):
                    for kk in range(16):
                        kh, kw = kk // 4, kk % 4
                        wt = wcp.tile([128, C], F32, tag="wcv")
                        nc.sync.dma_start(
                            wt[:], wconv_d[kk, ct * 128 : (ct + 1) * 128, :]
                        )
                        lhsT = xT_r[ct][:, 128 * pb : 128 * (pb + 1), kh * 4 + kw]
                        nc.tensor.matmul(
                            psc[:],
                            lhsT,
                            wt[:],
                            start=(idx == 0),
                            stop=False,
                            skip_group_check=True,
                        )
                        idx += 1
                nc.tensor.matmul(
                    psc[:],
                    ones_row[0:1, 0:128],
                    srb_sb[:],
                    start=False,
                    stop=True,
                    skip_group_check=True,
                )
                stats = small.tile([128, 6], F32, tag="st")
                nc.vector.bn_stats(stats[:], psc[:])
                mv = small.tile([128, 2], F32, tag="mv")
                nc.vector.bn_aggr(mv[:], stats[:])
                nmu = small.tile([128, 1], F32, tag="nmu")
                nc.vector.tensor_scalar(nmu[:], mv[:, 0:1], -1.0, None, ALU.mult)
                lv = small.tile([128, 1], F32, tag="lv")
                nc.scalar.activation(lv[:], mv[:, 1:2], AF.Ln, bias=eps_t[:], scale=1.0)
                rstd = small.tile([128, 1], F32, tag="rstd")
                nc.scalar.activation(rstd[:], lv[:], AF.Exp, bias=0.0, scale=-0.5)
                nc.vector.tensor_scalar(
                    ln_sb[pb][:], psc[:], nmu[:], rstd[:], ALU.add, ALU.mult
                )

            # lnT (feature-major LN output)
            for cb in range(2):
                ps = psc_p.tile([128, NK], F32, tag="psA")
                for pb in range(2):
                    nc.tensor.matmul(
                        ps[:, pb * 128 : (pb + 1) * 128],
                        ln_sb[pb][:, cb * 128 : (cb + 1) * 128],
                        ident[:],
                        is_transpose=True,
                        start=(pb == 0),
                        stop=(pb == 1),
                        skip_group_check=True,
                    )
                nc.vector.tensor_copy(lnT_sb[cb][:], ps[:])

            # kT feature-major per head-group
            for g in range(2):
                ps = psc_p.tile([128, NK], F32, tag="psA")
                for ct in range(2):
                    nc.tensor.matmul(
                        ps[:],
                        kvk_sb[ct][:, g * 128 : (g + 1) * 128],
                        lnT_sb[ct][:],
                        start=(ct == 0),
                        stop=(ct == 1),
                    )
                nc.vector.tensor_copy(kT_sb[g][:], ps[:])

            # v token-major
            for mb in range(2):
                ps = psc_p.tile([128, C], F32, tag="psA")
                for ct in range(2):
                    nc.tensor.matmul(
                        ps[:],
                        lnT_sb[ct][:, mb * 128 : (mb + 1) * 128],
                        kvv_sb[ct][:],
                        start=(ct == 0),
                        stop=(ct == 1),
                    )
                nc.vector.tensor_copy(v_sb[mb][:], ps[:])

            # qT feature-major
            psq = pa.enter_context(tc.tile_pool(name="ps_q", bufs=2, space="PSUM"))
            for g in range(2):
                for nt in range(NNT):
                    ps = psq.tile([128, NT], F32, tag="psq")
                    for ct in range(2):
                        nc.tensor.matmul(
                            ps[:],
                            qw_sb[ct][:, g * 128 : (g + 1) * 128],
                            xT_sb[ct][:, nt * NT : (nt + 1) * NT],
                            start=(ct == 0),
                            stop=(ct == 1),
                        )
                    nc.scalar.copy(qT_sb[g][:, nt * NT : (nt + 1) * NT], ps[:])

        # ---------------- Phase B: attention + proj ----------------
        with ExitStack() as pb_:
            expp = pb_.enter_context(tc.tile_pool(name="exp", bufs=4))
            aop = pb_.enter_context(tc.tile_pool(name="aoT", bufs=4))
            rsp = pb_.enter_context(tc.tile_pool(name="rs", bufs=2))
            outp = pb_.enter_context(tc.tile_pool(name="osb", bufs=4))
            ps_s = pb_.enter_context(tc.tile_pool(name="ps_s", bufs=2, space="PSUM"))
            ps_w = pb_.enter_context(tc.tile_pool(name="ps_w", bufs=4, space="PSUM"))

            for nt in range(NNT):
                aoT = {}
                for g in range(2):
                    ps_av = ps_w.tile([128, 512], F32, tag="work")
                    ps_cs = ps_w.tile([128, 512], F32, tag="work")
                    exp_sb = {}
                    for mb in range(2):
                        exp_t = expp.tile([128, 2048], F32, tag="exp")
                        for pair in range(2):
                            pss = ps_s.tile([128, 1024], F32, tag="scores")
                            for hh in range(2):
                                hp = pair * 2 + hh
                                nc.tensor.matmul(
                                    pss[:, hh * 512 : (hh + 1) * 512],
                                    kT_sb[g][32 * hp : 32 * (hp + 1), mb * 128 : (mb + 1) * 128],
                                    qT_sb[g][32 * hp : 32 * (hp + 1), nt * NT : (nt + 1) * NT],
                                    start=True,
                                    stop=True,
                                    tile_position=(32 * hp, 0),
                                )
                            nc.scalar.activation(
                                exp_t[:, pair * 1024 : (pair + 1) * 1024],
                                pss[:],
                                AF.Exp,
                                bias=0.0,
                                scale=1.0,
                            )
                        exp_sb[mb] = exp_t
                    # att @ v and column sums, accumulated over the two m-chunks
                    for mb in range(2):
                        for hp in range(4):
                            nc.tensor.matmul(
                                ps_av[32 * hp : 32 * (hp + 1), :],
                                v_sb[mb][:, 32 * (4 * g + hp) : 32 * (4 * g + hp + 1)],
                                exp_sb[mb][:, hp * 512 : (hp + 1) * 512],
                                start=(mb == 0),
                                stop=(mb == 1),
                                tile_position=(0, 32 * hp),
                                skip_group_check=True,
                            )
                            nc.tensor.matmul(
                                ps_cs[32 * hp : 32 * (hp + 1), :],
                                ones_row[:, 0:32],
                                exp_sb[mb][:, hp * 512 : (hp + 1) * 512],
                                start=(mb == 0),
                                stop=(mb == 1),
                                tile_position=(0, 32 * hp),
                                skip_group_check=True,
                            )
                    # 1/colsum, already broadcast across each 32-row head group
                    rs = rsp.tile([128, 512], F32, tag="rs")
                    nc.vector.reciprocal(rs[:], ps_cs[:])
                    ao = aop.tile([128, 512], F32, tag="ao")
                    nc.vector.tensor_mul(ao[:], ps_av[:], rs[:])
                    aoT[g] = ao

                # output projection, token-major
                for nb in range(4):
                    pso = ps_w.tile([128, 512], F32, tag="work")
                    for g in range(2):
                        nc.tensor.matmul(
                            pso[:, 0:C],
                            aoT[g][:, nb * 128 : (nb + 1) * 128],
                            projw_sb[g][:],
                            start=(g == 0),
                            stop=False,
                            skip_group_check=True,
                        )
                    nc.tensor.matmul(
                        pso[:, 0:C],
                        ones_row[0:1, 0:128],
                        projb_sb[:],
                        start=False,
                        stop=True,
                        skip_group_check=True,
                    )
                    osb = outp.tile([128, C], F32, tag="o")
                    nc.vector.tensor_copy(osb[:], pso[:, 0:C])
                    n0 = nt * NT + nb * 128
                    nc.sync.dma_start(out_d[n0 : n0 + 128, :], osb[:])


_NC_CACHE = None


def _get_nc():
    global _NC_CACHE
    if _NC_CACHE is None:
        nc = bacc.Bacc()
        x_d = nc.declare_dram_parameter("x", [N, C], F32, isOutput=False)
        qw_d = nc.declare_dram_parameter("qw", [C, C], F32, isOutput=False)
        kvk_d = nc.declare_dram_parameter("kvk", [C, C], F32, isOutput=False)
        kvv_d = nc.declare_dram_parameter("kvv", [C, C], F32, isOutput=False)
        wconv_d = nc.declare_dram_parameter("wconv", [16, C, C], F32, isOutput=False)
        srb_d = nc.declare_dram_parameter("srb", [1, C], F32, isOutput=False)
        projw_d = nc.declare_dram_parameter("projw", [C, C], F32, isOutput=False)
        projb_d = nc.declare_dram_parameter("projb", [1, C], F32, isOutput=False)
        out_d = nc.declare_dram_parameter("out", [N, C], F32, isOutput=True)
        with tile.TileContext(nc) as tc:
            _emit(tc, x_d, qw_d, kvk_d, kvv_d, wconv_d, srb_d, projw_d, projb_d, out_d)
        nc.compile()
        _NC_CACHE = nc
    return _NC_CACHE


def _prep(x, q_w, kv_w, sr_w, sr_b, ln_w, ln_b, proj_w, proj_b):
    x = np.asarray(x, np.float32)
    q_w = np.asarray(q_w, np.float32)
    kv_w = np.asarray(kv_w, np.float32)
    sr_w = np.asarray(sr_w, np.float32)
    sr_b = np.asarray(sr_b, np.float32)
    ln_w = np.asarray(ln_w, np.float32)
    ln_b = np.asarray(ln_b, np.float32)
    proj_w = np.asarray(proj_w, np.float32)
    proj_b = np.asarray(proj_b, np.float32)

    scale = float(HD) ** -0.5
    qw = np.ascontiguousarray(q_w * scale)
    # Fold ln_w into kv_w.  ln_b's effect on k is softmax-invariant (constant
    # along the softmax axis); its effect on v is a constant added to att@v
    # (attention rows sum to 1), folded into the projection bias.
    kvw = ln_w[:, None] * kv_w
    kvk = np.ascontiguousarray(kvw[:, :C])
    kvv = np.ascontiguousarray(kvw[:, C:])
    bv = ln_b @ kv_w[:, C:]
    projb = np.ascontiguousarray((proj_b + bv @ proj_w)[None, :])
    wconv = np.ascontiguousarray(sr_w.transpose(2, 3, 1, 0).reshape(16, C, C))
    srb = np.ascontiguousarray(sr_b[None, :])
    projw = np.ascontiguousarray(proj_w)

    return [
        dict(
            x=np.ascontiguousarray(x[b]),
            qw=qw,
            kvk=kvk,
            kvv=kvv,
            wconv=wconv,
            srb=srb,
            projw=projw,
            projb=projb,
        )
        for b in range(N_CORES)
    ]


def _run(in_maps, trace=False, **kwargs):
    nc = _get_nc()
    return run_bass_kernel_spmd(
        nc, in_maps, core_ids=list(range(N_CORES)), trace=trace, **kwargs
    )


def kernel(x, H, W, q_w, kv_w, sr_w, sr_b, ln_w, ln_b, proj_w, proj_b):
    assert int(H) == 64 and int(W) == 64
    in_maps = _prep(x, q_w, kv_w, sr_w, sr_b, ln_w, ln_b, proj_w, proj_b)
    res = _run(in_maps)
    out = np.stack([r["out"] for r in res.results], axis=0)
    return np.ascontiguousarray(out.astype(np.float32))


# revision 9
# speedup vs baseline: 1.9329x; 1.9329x over previous
"""Trainium2 Bass kernel for PVT-style spatial-reduction attention.

Reference computation (per batch element b):
    q   = x @ q_w                             [N=4096, C=256], heads=8, hd=32
    xs  = conv4x4_stride4(x as [C,64,64]) + sr_b   -> [Nk=256, C]
    xs  = LayerNorm(xs) (over C)
    kv  = xs @ kv_w -> k, v                   [Nk, C] each
    att = softmax(q @ k^T / sqrt(hd))         per head
    out = (att @ v) @ proj_w + proj_b         [N, C]

Sharding: pure data parallel -- batch B=8, one batch element per NeuronCore,
no collectives. Host folds ln_w/ln_b/scale into the weights and pre-casts
x and the weights to bf16 (PE matmuls accumulate in fp32 PSUM; fp32 matmul
on TRN2 runs as a 2-pass LOW/HIGH decomposition at half rate, so bf16
halves PE time; LN statistics, softmax input, and all output accumulation
stay fp32).

Device-side layout strategy (per core):
    xT  [C, N]   bf16 via hardware DMA transpose (2-byte dtypes only)
    xP  [C, N']  patch-major permuted copy (n' = i*256 + j*16 + a*4 + b) so
                 conv patch weights are single-strided APs
    qT  [C, N]   = q_w^T-contract (weights=q_w, rhs=xT)
    conv token-major [p, C] via 32 accumulated matmuls; +sr_b via K=1 matmul
    LN on free axis (bn_stats/bn_aggr); rsqrt = Exp(-0.5*Ln(var+eps))
    kT  [hd*4, Nk] per head-group (feature-major), v token-major [Nk, C]
    scores^T [m, n]: K=32 matmuls, 4 heads row-tiled (tile_position=(32h',0));
                 softmax needs no max-subtraction (|scores| < ~0.7)
    exp via ACT on [128, 1024] PSUM tiles -> bf16 SBUF
    colsum: col-tiled M=32 ones-matmuls (broadcast across the head group);
                 1/colsum as Exp(-Ln(x)) on ACT (reciprocal on DVE is 8x rate)
    att@v: col-tiled M=32 matmuls accumulating over m
    proj token-major (weights = aoT chunks) so the output DMA is contiguous
"""

from contextlib import ExitStack

import ml_dtypes
import numpy as np

import concourse.bass as bass
import concourse.tile as tile
from concourse import bacc, mybir
from concourse.bass_utils import run_bass_kernel_spmd
from concourse.masks import make_identity

F32 = mybir.dt.float32
BF16 = mybir.dt.bfloat16
AF = mybir.ActivationFunctionType
ALU = mybir.AluOpType

N_CORES = 8
N, C = 4096, 256
HEADS, HD = 8, 32
NK = 256  # (64/4) * (64/4) patches
LN_EPS = 1e-5
NT = 512  # n-tile width for the attention loop
NNT = N // NT


def _emit(tc, x_d, qw_d, kvk_d, kvv_d, wconv_d, srb_d, projw_d, projb_d, out_d):
    nc = tc.nc

    with ExitStack() as top:
        consts = top.enter_context(tc.tile_pool(name="consts", bufs=1))
        ident = consts.tile([128, 128], BF16)
        make_identity(nc, ident[:])
        ones_row = consts.tile([128, 128], BF16)
        nc.gpsimd.memset(ones_row[:], 1.0)
        eps_t = consts.tile([128, 1], F32)
        nc.gpsimd.memset(eps_t[:], LN_EPS)

        wpool = top.enter_context(tc.tile_pool(name="wts", bufs=1))

        def load_w(d, r0, tag):
            t = wpool.tile([128, C], BF16, tag=tag, name=tag)
            nc.sync.dma_start(t[:], d[r0 : r0 + 128, :])
            return t

        qw_sb = [load_w(qw_d, ct * 128, f"qw{ct}") for ct in range(2)]
        kvk_sb = [load_w(kvk_d, ct * 128, f"kvk{ct}") for ct in range(2)]
        kvv_sb = [load_w(kvv_d, ct * 128, f"kvv{ct}") for ct in range(2)]
        projw_sb = [load_w(projw_d, ct * 128, f"pw{ct}") for ct in range(2)]
        srb_sb = wpool.tile([1, C], BF16, tag="srb")
        nc.sync.dma_start(srb_sb[:], srb_d[:, :])
        projb_sb = wpool.tile([1, C], BF16, tag="pb")
        nc.sync.dma_start(projb_sb[:], projb_d[:, :])

        persist = top.enter_context(tc.tile_pool(name="persist", bufs=1))
        qT_sb = [persist.tile([128, N], BF16, tag=f"qT{g}", name=f"qT{g}") for g in range(2)]
        ln_sb = [persist.tile([128, C], BF16, tag=f"ln{p}", name=f"ln{p}") for p in range(2)]
        lnT_sb = [persist.tile([128, NK], BF16, tag=f"lnT{c}", name=f"lnT{c}") for c in range(2)]
        kT_sb = [persist.tile([128, NK], BF16, tag=f"kT{g}", name=f"kT{g}") for g in range(2)]
        v_sb = [persist.tile([128, C], BF16, tag=f"v{m}", name=f"v{m}") for m in range(2)]

        # ---------------- Phase A: xT, conv+LN, kT/v, qT ----------------
        with ExitStack() as pa:
            xTp = pa.enter_context(tc.tile_pool(name="xT", bufs=1))
            xT_sb = [xTp.tile([128, N], BF16, tag=f"xT{c}", name=f"xT{c}") for c in range(2)]
            xP_sb = [xTp.tile([128, N], BF16, tag=f"xP{c}", name=f"xP{c}") for c in range(2)]

            # xT via hardware DMA transpose (bf16 only path)
            for ct in range(2):
                nc.sync.dma_start(
                    xT_sb[ct][:], x_d[:, ct * 128 : (ct + 1) * 128], transpose=True
                )
            # xP[c, n'] with n' = i*256 + j*16 + a*4 + b (patch-major order)
            for ct in range(2):
                nc.vector.tensor_copy(
                    xP_sb[ct][:],
                    xT_sb[ct][:].rearrange(
                        "c (i a j b) -> c i j a b", i=16, a=4, j=16, b=4
                    ),
                )

            # conv (token-major) + LayerNorm
            wcp = pa.enter_context(tc.tile_pool(name="wconv", bufs=4))
            psc_p = pa.enter_context(tc.tile_pool(name="ps_c", bufs=2, space="PSUM"))
            small = pa.enter_context(tc.tile_pool(name="lnsmall", bufs=2))
            xT_r = [
                xP_sb[ct][:].rearrange("c (p s) -> c p s", p=256, s=16)
                for ct in range(2)
            ]
            for pb in range(2):
                psc = psc_p.tile([128, C], F32, tag="psA")
                idx = 0
                for ct in range(2):
                    for kk in range(16):
                        kh, kw = kk // 4, kk % 4
                        wt = wcp.tile([128, C], BF16, tag="wcv")
                        nc.sync.dma_start(
                            wt[:], wconv_d[kk, ct * 128 : (ct + 1) * 128, :]
                        )
                        lhsT = xT_r[ct][:, 128 * pb : 128 * (pb + 1), kh * 4 + kw]
                        nc.tensor.matmul(
                            psc[:],
                            lhsT,
                            wt[:],
                            start=(idx == 0),
                            stop=False,
                            skip_group_check=True,
                        )
                        idx += 1
                nc.tensor.matmul(
                    psc[:],
                    ones_row[0:1, 0:128],
                    srb_sb[:],
                    start=False,
                    stop=True,
                    skip_group_check=True,
                )
                stats = small.tile([128, 6], F32, tag="st")
                nc.vector.bn_stats(stats[:], psc[:])
                mv = small.tile([128, 2], F32, tag="mv")
                nc.vector.bn_aggr(mv[:], stats[:])
                nmu = small.tile([128, 1], F32, tag="nmu")
                nc.vector.tensor_scalar(nmu[:], mv[:, 0:1], -1.0, None, ALU.mult)
                lv = small.tile([128, 1], F32, tag="lv")
                nc.scalar.activation(lv[:], mv[:, 1:2], AF.Ln, bias=eps_t[:], scale=1.0)
                rstd = small.tile([128, 1], F32, tag="rstd")
                nc.scalar.activation(rstd[:], lv[:], AF.Exp, bias=0.0, scale=-0.5)
                nc.vector.tensor_scalar(
                    ln_sb[pb][:], psc[:], nmu[:], rstd[:], ALU.add, ALU.mult
                )

            # lnT (feature-major LN output) via PE transpose (bf16)
            for cb in range(2):
                ps = psc_p.tile([128, NK], BF16, tag="psT")
                for pb in range(2):
                    nc.tensor.matmul(
                        ps[:, pb * 128 : (pb + 1) * 128],
                        ln_sb[pb][:, cb * 128 : (cb + 1) * 128],
                        ident[:],
                        is_transpose=True,
                        start=(pb == 0),
                        stop=(pb == 1),
                        skip_group_check=True,
                    )
                nc.vector.tensor_copy(lnT_sb[cb][:], ps[:])

            # kT feature-major per head-group
            for g in range(2):
                ps = psc_p.tile([128, NK], F32, tag="psA")
                for ct in range(2):
                    nc.tensor.matmul(
                        ps[:],
                        kvk_sb[ct][:, g * 128 : (g + 1) * 128],
                        lnT_sb[ct][:],
                        start=(ct == 0),
                        stop=(ct == 1),
                    )
                nc.vector.tensor_copy(kT_sb[g][:], ps[:])

            # v token-major
            for mb in range(2):
                ps = psc_p.tile([128, C], F32, tag="psA")
                for ct in range(2):
                    nc.tensor.matmul(
                        ps[:],
                        lnT_sb[ct][:, mb * 128 : (mb + 1) * 128],
                        kvv_sb[ct][:],
                        start=(ct == 0),
                        stop=(ct == 1),
                    )
                nc.vector.tensor_copy(v_sb[mb][:], ps[:])

            # qT feature-major
            psq = pa.enter_context(tc.tile_pool(name="ps_q", bufs=2, space="PSUM"))
            for g in range(2):
                for nt in range(NNT):
                    ps = psq.tile([128, NT], F32, tag="psq")
                    for ct in range(2):
                        nc.tensor.matmul(
                            ps[:],
                            qw_sb[ct][:, g * 128 : (g + 1) * 128],
                            xT_sb[ct][:, nt * NT : (nt + 1) * NT],
                            start=(ct == 0),
                            stop=(ct == 1),
                        )
                    nc.scalar.copy(qT_sb[g][:, nt * NT : (nt + 1) * NT], ps[:])

        # ---------------- Phase B: attention + proj ----------------
        with ExitStack() as pb_:
            expp = pb_.enter_context(tc.tile_pool(name="exp", bufs=4))
            aop = pb_.enter_context(tc.tile_pool(name="aoT", bufs=4))
            rsp = pb_.enter_context(tc.tile_pool(name="rs", bufs=2))
            outp = pb_.enter_context(tc.tile_pool(name="osb", bufs=4))
            ps_s = pb_.enter_context(tc.tile_pool(name="ps_s", bufs=2, space="PSUM"))
            ps_w = pb_.enter_context(tc.tile_pool(name="ps_w", bufs=4, space="PSUM"))

            for nt in range(NNT):
                aoT = {}
                for g in range(2):
                    ps_av = ps_w.tile([128, 512], F32, tag="work")
                    ps_cs = ps_w.tile([128, 512], F32, tag="work")
                    exp_sb = {}
                    for mb in range(2):
                        exp_t = expp.tile([128, 2048], BF16, tag="exp")
                        for pair in range(2):
                            pss = ps_s.tile([128, 1024], F32, tag="scores")
                            for hh in range(2):
                                hp = pair * 2 + hh
                                nc.tensor.matmul(
                                    pss[:, hh * 512 : (hh + 1) * 512],
                                    kT_sb[g][32 * hp : 32 * (hp + 1), mb * 128 : (mb + 1) * 128],
                                    qT_sb[g][32 * hp : 32 * (hp + 1), nt * NT : (nt + 1) * NT],
                                    start=True,
                                    stop=True,
                                    tile_position=(32 * hp, 0),
                                )
                            nc.scalar.activation(
                                exp_t[:, pair * 1024 : (pair + 1) * 1024],
                                pss[:],
                                AF.Exp,
                                bias=0.0,
                                scale=1.0,
                            )
                        exp_sb[mb] = exp_t
                    # att @ v and column sums, accumulated over the two m-chunks
                    for mb in range(2):
                        for hp in range(4):
                            nc.tensor.matmul(
                                ps_av[32 * hp : 32 * (hp + 1), :],
                                v_sb[mb][:, 32 * (4 * g + hp) : 32 * (4 * g + hp + 1)],
                                exp_sb[mb][:, hp * 512 : (hp + 1) * 512],
                                start=(mb == 0),
                                stop=(mb == 1),
                                tile_position=(0, 32 * hp),
                                skip_group_check=True,
                            )
                            nc.tensor.matmul(
                                ps_cs[32 * hp : 32 * (hp + 1), :],
                                ones_row[:, 0:32],
                                exp_sb[mb][:, hp * 512 : (hp + 1) * 512],
                                start=(mb == 0),
                                stop=(mb == 1),
                                tile_position=(0, 32 * hp),
                                skip_group_check=True,
                            )
                    # 1/colsum on ACT via exp(-ln(x)); DVE reciprocal is 8 cyc/elem
                    lcs = rsp.tile([128, 512], F32, tag="lcs")
                    nc.scalar.activation(lcs[:], ps_cs[:], AF.Ln, bias=0.0, scale=1.0)
                    rs = rsp.tile([128, 512], F32, tag="rs")
                    nc.scalar.activation(rs[:], lcs[:], AF.Exp, bias=0.0, scale=-1.0)
                    ao = aop.tile([128, 512], BF16, tag="ao")
                    nc.vector.tensor_mul(ao[:], ps_av[:], rs[:])
                    aoT[g] = ao

                # output projection, token-major
                for nb in range(4):
                    pso = ps_w.tile([128, 512], F32, tag="work")
                    for g in range(2):
                        nc.tensor.matmul(
                            pso[:, 0:C],
                            aoT[g][:, nb * 128 : (nb + 1) * 128],
                            projw_sb[g][:],
                            start=(g == 0),
                            stop=False,
                            skip_group_check=True,
                        )
                    nc.tensor.matmul(
                        pso[:, 0:C],
                        ones_row[0:1, 0:128],
                        projb_sb[:],
                        start=False,
                        stop=True,
                        skip_group_check=True,
                    )
                    osb = outp.tile([128, C], F32, tag="o")
                    nc.vector.tensor_copy(osb[:], pso[:, 0:C])
                    n0 = nt * NT + nb * 128
                    nc.sync.dma_start(out_d[n0 : n0 + 128, :], osb[:])


_NC_CACHE = None


def _get_nc():
    global _NC_CACHE
    if _NC_CACHE is None:
        nc = bacc.Bacc()
        x_d = nc.declare_dram_parameter("x", [N, C], BF16, isOutput=False)
        qw_d = nc.declare_dram_parameter("qw", [C, C], BF16, isOutput=False)
        kvk_d = nc.declare_dram_parameter("kvk", [C, C], BF16, isOutput=False)
        kvv_d = nc.declare_dram_parameter("kvv", [C, C], BF16, isOutput=False)
        wconv_d = nc.declare_dram_parameter("wconv", [16, C, C], BF16, isOutput=False)
        srb_d = nc.declare_dram_parameter("srb", [1, C], BF16, isOutput=False)
        projw_d = nc.declare_dram_parameter("projw", [C, C], BF16, isOutput=False)
        projb_d = nc.declare_dram_parameter("projb", [1, C], BF16, isOutput=False)
        out_d = nc.declare_dram_parameter("out", [N, C], F32, isOutput=True)
        with tile.TileContext(nc) as tc:
            _emit(tc, x_d, qw_d, kvk_d, kvv_d, wconv_d, srb_d, projw_d, projb_d, out_d)
        nc.compile()
        _NC_CACHE = nc
    return _NC_CACHE


def _bf16(a):
    return np.ascontiguousarray(np.asarray(a, np.float32).astype(ml_dtypes.bfloat16))


def _prep(x, q_w, kv_w, sr_w, sr_b, ln_w, ln_b, proj_w, proj_b):
    x = np.asarray(x, np.float32)
    q_w = np.asarray(q_w, np.float32)
    kv_w = np.asarray(kv_w, np.float32)
    sr_w = np.asarray(sr_w, np.float32)
    sr_b = np.asarray(sr_b, np.float32)
    ln_w = np.asarray(ln_w, np.float32)
    ln_b = np.asarray(ln_b, np.float32)
    proj_w = np.asarray(proj_w, np.float32)
    proj_b = np.asarray(proj_b, np.float32)

    scale = float(HD) ** -0.5
    qw = _bf16(q_w * scale)
    # Fold ln_w into kv_w.  ln_b's effect on k is softmax-invariant (constant
    # along the softmax axis); its effect on v is a constant added to att@v
    # (attention rows sum to 1), folded into the projection bias.
    kvw = ln_w[:, None] * kv_w
    kvk = _bf16(kvw[:, :C])
    kvv = _bf16(kvw[:, C:])
    bv = ln_b @ kv_w[:, C:]
    projb = _bf16((proj_b + bv @ proj_w)[None, :])
    wconv = _bf16(sr_w.transpose(2, 3, 1, 0).reshape(16, C, C))
    srb = _bf16(sr_b[None, :])
    projw = _bf16(proj_w)

    return [
        dict(
            x=_bf16(x[b]),
            qw=qw,
            kvk=kvk,
            kvv=kvv,
            wconv=wconv,
            srb=srb,
            projw=projw,
            projb=projb,
        )
        for b in range(N_CORES)
    ]


def _run(in_maps, trace=False, **kwargs):
    nc = _get_nc()
    return run_bass_kernel_spmd(
        nc, in_maps, core_ids=list(range(N_CORES)), trace=trace, **kwargs
    )


def kernel(x, H, W, q_w, kv_w, sr_w, sr_b, ln_w, ln_b, proj_w, proj_b):
    assert int(H) == 64 and int(W) == 64
    in_maps = _prep(x, q_w, kv_w, sr_w, sr_b, ln_w, ln_b, proj_w, proj_b)
    res = _run(in_maps)
    out = np.stack([r["out"] for r in res.results], axis=0)
    return np.ascontiguousarray(out.astype(np.float32))


# revision 13
# speedup vs baseline: 2.2790x; 1.1791x over previous
"""Trainium2 Bass kernel for PVT-style spatial-reduction attention.

Reference computation (per batch element b):
    q   = x @ q_w                             [N=4096, C=256], heads=8, hd=32
    xs  = conv4x4_stride4(x as [C,64,64]) + sr_b   -> [Nk=256, C]
    xs  = LayerNorm(xs) (over C)
    kv  = xs @ kv_w -> k, v                   [Nk, C] each
    att = softmax(q @ k^T / sqrt(hd))         per head
    out = (att @ v) @ proj_w + proj_b         [N, C]

Sharding: pure data parallel -- batch B=8, one batch element per NeuronCore,
no collectives. Host folds ln_w/ln_b/scale into the weights and pre-casts
x and the weights to bf16 (PE matmuls accumulate in fp32 PSUM; fp32 matmul
on TRN2 runs as a 2-pass LOW/HIGH decomposition at half rate, so bf16
halves PE time; LN statistics, softmax input, and all output accumulation
stay fp32).

Device-side layout strategy (per core):
    xT  [C, N]   bf16 via hardware DMA transpose (2-byte dtypes only)
    xP  [C, N']  patch-major permuted copy (n' = i*256 + j*16 + a*4 + b) so
                 conv patch weights are single-strided APs
    qT  [C, N]   = q_w^T-contract (weights=q_w, rhs=xT)
    conv token-major [p, C] via 32 accumulated matmuls; +sr_b via K=1 matmul
    LN on free axis (bn_stats/bn_aggr); rsqrt = Exp(-0.5*Ln(var+eps))
    kT  [hd*4, Nk] per head-group (feature-major), v token-major [Nk, C]
    scores^T [m, n]: K=32 matmuls, 4 heads row-tiled (tile_position=(32h',0));
                 softmax needs no max-subtraction (|scores| < ~0.7)
    exp via ACT on [128, 1024] PSUM tiles -> bf16 SBUF
    colsum: col-tiled M=32 ones-matmuls (broadcast across the head group);
                 1/colsum as Exp(-Ln(x)) on ACT (reciprocal on DVE is 8x rate)
    att@v: col-tiled M=32 matmuls accumulating over m
    proj token-major (weights = aoT chunks) so the output DMA is contiguous
"""

from contextlib import ExitStack

import ml_dtypes
import numpy as np

import concourse.bass as bass
import concourse.tile as tile
from concourse import bacc, mybir
from concourse.bass_utils import run_bass_kernel_spmd
from concourse.masks import make_identity

F32 = mybir.dt.float32
BF16 = mybir.dt.bfloat16
AF = mybir.ActivationFunctionType
ALU = mybir.AluOpType

N_CORES = 8
N, C = 4096, 256
HEADS, HD = 8, 32
NK = 256  # (64/4) * (64/4) patches
LN_EPS = 1e-5
NT = 512  # n-tile width for the attention loop
NNT = N // NT


def _emit(tc, x_d, qw_d, kvk_d, kvv_d, wconv_d, srb_d, projw_d, projb_d, out_d):
    nc = tc.nc

    with ExitStack() as top:
        consts = top.enter_context(tc.tile_pool(name="consts", bufs=1))
        ident = consts.tile([128, 128], BF16)
        make_identity(nc, ident[:])
        ones_row = consts.tile([128, 128], BF16)
        nc.gpsimd.memset(ones_row[:], 1.0)
        eps_t = consts.tile([128, 1], F32)
        nc.gpsimd.memset(eps_t[:], LN_EPS)

        wpool = top.enter_context(tc.tile_pool(name="wts", bufs=1))

        def load_w(d, r0, tag):
            t = wpool.tile([128, C], BF16, tag=tag, name=tag)
            nc.sync.dma_start(t[:], d[r0 : r0 + 128, :])
            return t

        qw_sb = [load_w(qw_d, ct * 128, f"qw{ct}") for ct in range(2)]
        kvk_sb = [load_w(kvk_d, ct * 128, f"kvk{ct}") for ct in range(2)]
        kvv_sb = [load_w(kvv_d, ct * 128, f"kvv{ct}") for ct in range(2)]
        projw_sb = [load_w(projw_d, ct * 128, f"pw{ct}") for ct in range(2)]
        srb_sb = wpool.tile([1, C], BF16, tag="srb")
        nc.sync.dma_start(srb_sb[:], srb_d[:, :])
        projb_sb = wpool.tile([1, C], BF16, tag="pb")
        nc.sync.dma_start(projb_sb[:], projb_d[:, :])

        persist = top.enter_context(tc.tile_pool(name="persist", bufs=1))
        qT_sb = [persist.tile([128, N], BF16, tag=f"qT{g}", name=f"qT{g}") for g in range(2)]
        ln_sb = [persist.tile([128, C], BF16, tag=f"ln{p}", name=f"ln{p}") for p in range(2)]
        lnT_sb = [persist.tile([128, NK], BF16, tag=f"lnT{c}", name=f"lnT{c}") for c in range(2)]
        kT_sb = [persist.tile([128, NK], BF16, tag=f"kT{g}", name=f"kT{g}") for g in range(2)]
        v_sb = [persist.tile([128, C], BF16, tag=f"v{m}", name=f"v{m}") for m in range(2)]

        # ---------------- Phase A: xT, conv+LN, kT/v, qT ----------------
        with ExitStack() as pa:
            xTp = pa.enter_context(tc.tile_pool(name="xT", bufs=1))
            xT_sb = [xTp.tile([128, N], BF16, tag=f"xT{c}", name=f"xT{c}") for c in range(2)]
            xP_sb = [xTp.tile([128, N], BF16, tag=f"xP{c}", name=f"xP{c}") for c in range(2)]

            # xT via hardware DMA transpose (bf16 only path)
            for ct in range(2):
                nc.sync.dma_start(
                    xT_sb[ct][:], x_d[:, ct * 128 : (ct + 1) * 128], transpose=True
                )
            # xP[c, n'] with n' = i*256 + j*16 + a*4 + b (patch-major order)
            for ct in range(2):
                nc.vector.tensor_copy(
                    xP_sb[ct][:],
                    xT_sb[ct][:].rearrange(
                        "c (i a j b) -> c i j a b", i=16, a=4, j=16, b=4
                    ),
                )

            # conv (token-major) + LayerNorm
            wcp = pa.enter_context(tc.tile_pool(name="wconv", bufs=4))
            psc_p = pa.enter_context(tc.tile_pool(name="ps_c", bufs=2, space="PSUM"))
            small = pa.enter_context(tc.tile_pool(name="lnsmall", bufs=2))
            xT_r = [
                xP_sb[ct][:].rearrange("c (p s) -> c p s", p=256, s=16)
                for ct in range(2)
            ]
            for pb in range(2):
                psc = psc_p.tile([128, C], F32, tag="psA")
                idx = 0
                for ct in range(2):
                    for kk in range(16):
                        kh, kw = kk // 4, kk % 4
                        wt = wcp.tile([128, C], BF16, tag="wcv")
                        nc.sync.dma_start(
                            wt[:], wconv_d[kk, ct * 128 : (ct + 1) * 128, :]
                        )
                        lhsT = xT_r[ct][:, 128 * pb : 128 * (pb + 1), kh * 4 + kw]
                        nc.tensor.matmul(
                            psc[:],
                            lhsT,
                            wt[:],
                            start=(idx == 0),
                            stop=False,
                            skip_group_check=True,
                        )
                        idx += 1
                nc.tensor.matmul(
                    psc[:],
                    ones_row[0:1, 0:128],
                    srb_sb[:],
                    start=False,
                    stop=True,
                    skip_group_check=True,
                )
                stats = small.tile([128, 6], F32, tag="st")
                nc.vector.bn_stats(stats[:], psc[:])
                mv = small.tile([128, 2], F32, tag="mv")
                nc.vector.bn_aggr(mv[:], stats[:])
                nmu = small.tile([128, 1], F32, tag="nmu")
                nc.vector.tensor_scalar(nmu[:], mv[:, 0:1], -1.0, None, ALU.mult)
                lv = small.tile([128, 1], F32, tag="lv")
                nc.scalar.activation(lv[:], mv[:, 1:2], AF.Ln, bias=eps_t[:], scale=1.0)
                rstd = small.tile([128, 1], F32, tag="rstd")
                nc.scalar.activation(rstd[:], lv[:], AF.Exp, bias=0.0, scale=-0.5)
                nc.vector.tensor_scalar(
                    ln_sb[pb][:], psc[:], nmu[:], rstd[:], ALU.add, ALU.mult
                )

            # lnT (feature-major LN output) via PE transpose (bf16)
            for cb in range(2):
                ps = psc_p.tile([128, NK], BF16, tag="psT")
                for pb in range(2):
                    nc.tensor.matmul(
                        ps[:, pb * 128 : (pb + 1) * 128],
                        ln_sb[pb][:, cb * 128 : (cb + 1) * 128],
                        ident[:],
                        is_transpose=True,
                        start=(pb == 0),
                        stop=(pb == 1),
                        skip_group_check=True,
                    )
                nc.vector.tensor_copy(lnT_sb[cb][:], ps[:])

            # kT feature-major per head-group
            for g in range(2):
                ps = psc_p.tile([128, NK], F32, tag="psA")
                for ct in range(2):
                    nc.tensor.matmul(
                        ps[:],
                        kvk_sb[ct][:, g * 128 : (g + 1) * 128],
                        lnT_sb[ct][:],
                        start=(ct == 0),
                        stop=(ct == 1),
                    )
                nc.vector.tensor_copy(kT_sb[g][:], ps[:])

            # v token-major
            for mb in range(2):
                ps = psc_p.tile([128, C], F32, tag="psA")
                for ct in range(2):
                    nc.tensor.matmul(
                        ps[:],
                        lnT_sb[ct][:, mb * 128 : (mb + 1) * 128],
                        kvv_sb[ct][:],
                        start=(ct == 0),
                        stop=(ct == 1),
                    )
                nc.vector.tensor_copy(v_sb[mb][:], ps[:])

            # qT feature-major
            psq = pa.enter_context(tc.tile_pool(name="ps_q", bufs=2, space="PSUM"))
            for g in range(2):
                for nt in range(NNT):
                    ps = psq.tile([128, NT], F32, tag="psq")
                    for ct in range(2):
                        nc.tensor.matmul(
                            ps[:],
                            qw_sb[ct][:, g * 128 : (g + 1) * 128],
                            xT_sb[ct][:, nt * NT : (nt + 1) * NT],
                            start=(ct == 0),
                            stop=(ct == 1),
                        )
                    nc.vector.tensor_copy(qT_sb[g][:, nt * NT : (nt + 1) * NT], ps[:])

        # ---------------- Phase B: attention + proj ----------------
        with ExitStack() as pb_:
            expp = pb_.enter_context(tc.tile_pool(name="exp", bufs=6))
            aop = pb_.enter_context(tc.tile_pool(name="aoT", bufs=6))
            rsp = pb_.enter_context(tc.tile_pool(name="rs", bufs=4))
            outp = pb_.enter_context(tc.tile_pool(name="osb", bufs=6))
            ps_s = pb_.enter_context(tc.tile_pool(name="ps_s", bufs=2, space="PSUM"))
            ps_w = pb_.enter_context(tc.tile_pool(name="ps_w", bufs=4, space="PSUM"))

            for nt in range(NNT):
                aoT = {}
                for g in range(2):
                    ps_av = ps_w.tile([128, 512], F32, tag="work")
                    ps_cs = ps_w.tile([128, 512], F32, tag="work")
                    exp_sb = {}
                    for mb in range(2):
                        exp_t = expp.tile([128, 2048], BF16, tag="exp")
                        for pair in range(2):
                            pss = ps_s.tile([128, 1024], F32, tag="scores")
                            for hh in range(2):
                                hp = pair * 2 + hh
                                nc.tensor.matmul(
                                    pss[:, hh * 512 : (hh + 1) * 512],
                                    kT_sb[g][32 * hp : 32 * (hp + 1), mb * 128 : (mb + 1) * 128],
                                    qT_sb[g][32 * hp : 32 * (hp + 1), nt * NT : (nt + 1) * NT],
                                    start=True,
                                    stop=True,
                                    tile_position=(32 * hp, 0),
                                )
                            nc.scalar.activation(
                                exp_t[:, pair * 1024 : (pair + 1) * 1024],
                                pss[:],
                                AF.Exp,
                                bias=0.0,
                                scale=1.0,
                            )
                        exp_sb[mb] = exp_t
                    # att @ v and column sums, accumulated over the two m-chunks
                    for mb in range(2):
                        for hp in range(4):
                            nc.tensor.matmul(
                                ps_av[32 * hp : 32 * (hp + 1), :],
                                v_sb[mb][:, 32 * (4 * g + hp) : 32 * (4 * g + hp + 1)],
                                exp_sb[mb][:, hp * 512 : (hp + 1) * 512],
                                start=(mb == 0),
                                stop=(mb == 1),
                                tile_position=(0, 32 * hp),
                                skip_group_check=True,
                            )
                            nc.tensor.matmul(
                                ps_cs[32 * hp : 32 * (hp + 1), :],
                                ones_row[:, 0:32],
                                exp_sb[mb][:, hp * 512 : (hp + 1) * 512],
                                start=(mb == 0),
                                stop=(mb == 1),
                                tile_position=(0, 32 * hp),
                                skip_group_check=True,
                            )
                    # 1/colsum: fast-approx reciprocal (single DVE op, ~51 ULP;
                    # avoids ACT-table thrash between Ln and Exp sets)
                    rs = rsp.tile([128, 512], F32, tag="rs")
                    nc.vector.reciprocal_approx_fast(out=rs[:], in_=ps_cs[:])
                    ao = aop.tile([128, 512], BF16, tag="ao")
                    nc.vector.tensor_mul(ao[:], ps_av[:], rs[:])
                    aoT[g] = ao

                # output projection, token-major
                for nb in range(4):
                    pso = ps_w.tile([128, 512], F32, tag="work")
                    for g in range(2):
                        nc.tensor.matmul(
                            pso[:, 0:C],
                            aoT[g][:, nb * 128 : (nb + 1) * 128],
                            projw_sb[g][:],
                            start=(g == 0),
                            stop=False,
                            skip_group_check=True,
                        )
                    nc.tensor.matmul(
                        pso[:, 0:C],
                        ones_row[0:1, 0:128],
                        projb_sb[:],
                        start=False,
                        stop=True,
                        skip_group_check=True,
                    )
                    osb = outp.tile([128, C], F32, tag="o")
                    nc.vector.tensor_copy(osb[:], pso[:, 0:C])
                    n0 = nt * NT + nb * 128
                    nc.sync.dma_start(out_d[n0 : n0 + 128, :], osb[:])


_NC_CACHE = None


def _get_nc():
    global _NC_CACHE
    if _NC_CACHE is None:
        nc = bacc.Bacc()
        x_d = nc.declare_dram_parameter("x", [N, C], BF16, isOutput=False)
        qw_d = nc.declare_dram_parameter("qw", [C, C], BF16, isOutput=False)
        kvk_d = nc.declare_dram_parameter("kvk", [C, C], BF16, isOutput=False)
        kvv_d = nc.declare_dram_parameter("kvv", [C, C], BF16, isOutput=False)
        wconv_d = nc.declare_dram_parameter("wconv", [16, C, C], BF16, isOutput=False)
        srb_d = nc.declare_dram_parameter("srb", [1, C], BF16, isOutput=False)
        projw_d = nc.declare_dram_parameter("projw", [C, C], BF16, isOutput=False)
        projb_d = nc.declare_dram_parameter("projb", [1, C], BF16, isOutput=False)
        out_d = nc.declare_dram_parameter("out", [N, C], F32, isOutput=True)
        with tile.TileContext(nc) as tc:
            _emit(tc, x_d, qw_d, kvk_d, kvv_d, wconv_d, srb_d, projw_d, projb_d, out_d)
        nc.compile()
        _NC_CACHE = nc
    return _NC_CACHE


def _bf16(a):
    return np.ascontiguousarray(np.asarray(a, np.float32).astype(ml_dtypes.bfloat16))


def _prep(x, q_w, kv_w, sr_w, sr_b, ln_w, ln_b, proj_w, proj_b):
    x = np.asarray(x, np.float32)
    q_w = np.asarray(q_w, np.float32)
    kv_w = np.asarray(kv_w, np.float32)
    sr_w = np.asarray(sr_w, np.float32)
    sr_b = np.asarray(sr_b, np.float32)
    ln_w = np.asarray(ln_w, np.float32)
    ln_b = np.asarray(ln_b, np.float32)
    proj_w = np.asarray(proj_w, np.float32)
    proj_b = np.asarray(proj_b, np.float32)

    scale = float(HD) ** -0.5
    qw = _bf16(q_w * scale)
    # Fold ln_w into kv_w.  ln_b's effect on k is softmax-invariant (constant
    # along the softmax axis); its effect on v is a constant added to att@v
    # (attention rows sum to 1), folded into the projection bias.
    kvw = ln_w[:, None] * kv_w
    kvk = _bf16(kvw[:, :C])
    kvv = _bf16(kvw[:, C:])
    bv = ln_b @ kv_w[:, C:]
    projb = _bf16((proj_b + bv @ proj_w)[None, :])
    wconv = _bf16(sr_w.transpose(2, 3, 1, 0).reshape(16, C, C))
    srb = _bf16(sr_b[None, :])
    projw = _bf16(proj_w)

    return [
        dict(
            x=_bf16(x[b]),
            qw=qw,
            kvk=kvk,
            kvv=kvv,
            wconv=wconv,
            srb=srb,
            projw=projw,
            projb=projb,
        )
        for b in range(N_CORES)
    ]


def _run(in_maps, trace=False, **kwargs):
    nc = _get_nc()
    return run_bass_kernel_spmd(
        nc, in_maps, core_ids=list(range(N_CORES)), trace=trace, **kwargs
    )


def kernel(x, H, W, q_w, kv_w, sr_w, sr_b, ln_w, ln_b, proj_w, proj_b):
    assert int(H) == 64 and int(W) == 64
    in_maps = _prep(x, q_w, kv_w, sr_w, sr_b, ln_w, ln_b, proj_w, proj_b)
    res = _run(in_maps)
    out = np.stack([r["out"] for r in res.results], axis=0)
    return np.ascontiguousarray(out.astype(np.float32))
